# revision 62
# baseline (speedup 1.0000x reference)
"""Distributed Trainium2 kernel for LayerNorm -> biased multi-head attention -> out-proj.

Problem shapes (hardcoded):
  x        [4, 2048, 1024] f32
  attn_bias[16, 2048, 2048] f32
  ln_g/ln_b[1024] f32
  Wq       [1024, 1024] f32
  Wkv      [1024, 2048] f32
  Wout     [1024, 1024] f32
  out      [4, 2048, 1024] f32

Sharding: sequence-sharded over 8 cores; core r owns query rows
[r*256, (r+1)*256) of every batch. Host-side prep (layout only): x and the
weights are cast to bf16; attn_bias is sliced per core, transposed to
kv-major [H, 128, KC*256] bf16 (kv = c*128 + p) so the device consumes it
directly with contiguous 8KB DMA lines and a single exp() per head.

Per core: bf16 LN + q/k/v for its rows; k^T (inner-major) and v
(token-major) land in one kv_loc tensor (batched strided v-stores) and are
AllGathered in bf16 as 8 per-head-pair pieces so attention on pair i starts
as soon as piece i has arrived; qT projections are interleaved with the k
pieces so pair 0 is ready early. Softmax = exp(sim)*exp(biasT) (no max
subtraction; values are small); exp(biasT) tiles are prefetched two pairs
ahead on the gpsimd ring so their pool-buffer waits never block the ACT
queue (ACT = the exp(sim) critical engine). The softmax denominator comes
from 4-way column-tiled ones-weight matmuls into one [97,512] PSUM tile;
attn@v packs the two heads of a pair onto PE column halves via
tile_position. Per-pair reciprocals are stashed (2 pairs per [97,1024]
tile) and the recip-broadcast matmul + normalize of pair i runs one pair
late, keeping the PE queue free of DVE round-trips; out-proj runs at the
tail. Engine/queue split: sync = x + v/kt loads + out, scalar(ACT) = exps +
k-stores, gpsimd = weights/vt/ebt/wout + collectives. PSUM budget (8
banks): sim 2x2 + po 2 + dn 1 + rb 1. PSUM accumulators are
zero-initialized with start=True zero-weight matmuls so interleaved
accumulation groups sharing a bank never clear each other's has_written
bits.
"""

import numpy as np

CORES = 8
B = 4
N = 2048
NLOC = N // CORES          # 256
ROWS = B * NLOC            # 1024 local query rows (row = b*NLOC + q)
DIM = 1024
H = 16
D = 64
PAIRS = H // 2             # head pairs
KC = N // 128              # 16 kv chunks of 128 tokens per batch
KP = 128 * DIM             # k^T part of one AG piece (elements)
VP = ROWS * 128            # v part of one AG piece
PIECE = KP + VP            # per-rank payload of one piece (bf16 elements)
SCALE = D ** -0.5
EPS = 1e-5

_CACHE = {}


def _build_nc():
    import contextlib
    import concourse.bass as bass
    import concourse.bacc as bacc
    import concourse.tile as tile
    import concourse.mybir as mybir
    from concourse import masks

    f32 = mybir.dt.float32
    bf16 = mybir.dt.bfloat16
    AF = mybir.ActivationFunctionType
    ALU = mybir.AluOpType

    nc = bacc.Bacc("TRN2", target_bir_lowering=False, debug=False,
                   num_devices=CORES)

    x_in = nc.dram_tensor("x", [ROWS, DIM], bf16, kind="ExternalInput")
    # pre-transposed kv-major bias, bf16: [h][p][c*256+q], kv = c*128+p
    bias_in = nc.dram_tensor("attn_bias", [H, 128, KC * NLOC], bf16,
                             kind="ExternalInput")
    ln_g = nc.dram_tensor("ln_g", [DIM], bf16, kind="ExternalInput")
    ln_b = nc.dram_tensor("ln_b", [DIM], bf16, kind="ExternalInput")
    wq_in = nc.dram_tensor("Wq", [DIM, DIM], bf16, kind="ExternalInput")
    wkv_in = nc.dram_tensor("Wkv", [DIM, 2 * DIM], bf16, kind="ExternalInput")
    wout_in = nc.dram_tensor("Wout", [DIM, DIM], bf16, kind="ExternalInput")
    out_ext = nc.dram_tensor("out", [ROWS, DIM], f32, kind="ExternalOutput")

    with tile.TileContext(nc) as tc, contextlib.ExitStack() as top:
        # ------------------------------------------------------------------
        # DRAM scratch
        dram = top.enter_context(tc.tile_pool(name="dram", bufs=1, space="DRAM"))
        kv_loc = dram.tile([PAIRS * PIECE], bf16, name="kvl")
        kv_ful = [dram.tile([CORES * PIECE], bf16, name=f"kvf{i}",
                            addr_space="Shared") for i in range(PAIRS)]

        # ------------------------------------------------------------------
        # Constants
        cpool = top.enter_context(tc.tile_pool(name="consts", bufs=1))
        identity_bf = cpool.tile([128, 128], bf16, name="identity_bf")
        masks.make_identity(nc, identity_bf[:])
        eps_t = cpool.tile([128, 1], f32, name="eps_t")
        nc.vector.memset(eps_t[:], EPS)
        # selector rows for denominator broadcast: rows 0/64 mask PE cols
        # 0-63 (head-even), rows 32/96 mask cols 64-127 (head-odd)
        sel_full = cpool.tile([97, 128], f32, name="sel_full")
        ones64 = cpool.tile([32, D], f32, name="ones64")
        nc.gpsimd.memset(sel_full[:], 0.0)
        nc.gpsimd.memset(ones64[:], 1.0)
        nc.gpsimd.dma_start(sel_full[0:1, 0:D], ones64[0:1, :])
        nc.gpsimd.dma_start(sel_full[32:33, D:2 * D], ones64[0:1, :])
        nc.gpsimd.dma_start(sel_full[64:65, 0:D], ones64[0:1, :])
        nc.gpsimd.dma_start(sel_full[96:97, D:2 * D], ones64[0:1, :])
        zeros128 = cpool.tile([128, 128], bf16, name="zeros128")
        nc.vector.memset(zeros128[:], 0.0)
        ones1 = cpool.tile([128, 1], bf16, name="ones1")
        nc.vector.memset(ones1[:], 1.0)

        # ------------------------------------------------------------------
        # Persistent pools (live until the end; LIFO with the top stack)
        qt_pool = top.enter_context(tc.tile_pool(name="qT", bufs=1))
        asm_pool = top.enter_context(tc.tile_pool(name="asm", bufs=1))
        asm = [asm_pool.tile([128, ROWS], bf16, name=f"asm{i}") for i in range(8)]
        wo_pool = top.enter_context(tc.tile_pool(name="wo", bufs=1))
        dk_pool = top.enter_context(tc.tile_pool(name="dkeep", bufs=1))
        den_keep = [dk_pool.tile([97, ROWS], f32, name=f"dk{j}")
                    for j in range(4)]
        for j in range(4):
            nc.gpsimd.memset(den_keep[j][:], 1.0)

        # biasT tiles [128 p, KC*256] bf16 per head: load + exp in-place.
        # Opened early so pairs 0-2 exp during the LN/QKV window while the
        # scalar engine is otherwise idle.
        ebt_stack = contextlib.ExitStack()
        ebt_pool = ebt_stack.enter_context(tc.tile_pool(name="ebT", bufs=6))
        ebt_tiles = {}

        def load_ebt(h):
            et = ebt_pool.tile([128, KC * NLOC], bf16, name="ebt")
            nc.gpsimd.dma_start(et[:], bias_in[h])
            nc.scalar.activation(et[:], et[:], AF.Exp)
            ebt_tiles[h] = et

        # ------------------------------------------------------------------
        # Weights: bf16 direct HWDGE loads (cast done host-side).
        w_stack = contextlib.ExitStack()
        wq_pool = w_stack.enter_context(tc.tile_pool(name="wq", bufs=1))
        wkv_pool = w_stack.enter_context(tc.tile_pool(name="wkv", bufs=1))
        wq_bf, wkv_bf = [], []
        for t in range(8):
            wkt = wkv_pool.tile([128, 2 * DIM], bf16, name=f"wkv{t}")
            nc.gpsimd.dma_start(wkt[:], wkv_in[t * 128:(t + 1) * 128, :])
            wkv_bf.append(wkt)
        for t in range(8):
            wqt = wq_pool.tile([128, DIM], bf16, name=f"wq{t}")
            nc.gpsimd.dma_start(wqt[:], wq_in[t * 128:(t + 1) * 128, :])
            wq_bf.append(wqt)

        for h in range(6):
            load_ebt(h)

        # xnT pool opened before the LN pool so LN tiles can be freed first
        xnt_stack = contextlib.ExitStack()
        xnt_pool = xnt_stack.enter_context(tc.tile_pool(name="xnT", bufs=1))

        # ------------------------------------------------------------------
        # Phase 1: LayerNorm (rows on partitions) -> xn f32 in place
        ln_pool = contextlib.ExitStack()
        xpool = ln_pool.enter_context(tc.tile_pool(name="x", bufs=1))
        spool = ln_pool.enter_context(tc.tile_pool(name="stats", bufs=1))
        g_t = spool.tile([128, DIM], bf16, name="g_t")
        b_t = spool.tile([128, DIM], bf16, name="b_t")
        nc.sync.dma_start(
            out=g_t[:],
            in_=bass.AP(tensor=ln_g.ap().tensor, offset=0, ap=[[0, 128], [1, DIM]]))
        nc.sync.dma_start(
            out=b_t[:],
            in_=bass.AP(tensor=ln_b.ap().tensor, offset=0, ap=[[0, 128], [1, DIM]]))
        x_t = []
        for s in range(8):
            xt = xpool.tile([128, DIM], bf16, name=f"x{s}")
            nc.sync.dma_start(xt[:], x_in[s * 128:(s + 1) * 128, :])
            stats = spool.tile([128, 2, 6], f32, name=f"st{s}")
            mv = spool.tile([128, 2], f32, name=f"mv{s}")
            for g in range(2):
                nc.vector.bn_stats(stats[:, g], xt[:, g * 512:(g + 1) * 512])
            nc.vector.bn_aggr(mv[:], stats[:])
            # rstd = 1/sqrt(var + eps)
            nc.scalar.activation(mv[:, 1:2], mv[:, 1:2], AF.Sqrt,
                                 bias=eps_t[:, 0:1], scale=1.0)
            nc.vector.reciprocal(mv[:, 1:2], mv[:, 1:2])
            nc.vector.tensor_scalar(out=xt[:], in0=xt[:],
                                    scalar1=mv[:, 0:1], scalar2=mv[:, 1:2],
                                    op0=ALU.subtract, op1=ALU.mult)
            nc.vector.tensor_mul(xt[:], xt[:], g_t[:])
            nc.vector.tensor_add(xt[:], xt[:], b_t[:])
            x_t.append(xt)

        # ------------------------------------------------------------------
        # Phase 2: transpose xn -> xnT bf16 [dim-part, row-free]
        tr_stack = contextlib.ExitStack()
        tr_pool = tr_stack.enter_context(
            tc.tile_pool(name="trps", bufs=2, space="PSUM"))
        xnT = []
        for t in range(8):
            ps = tr_pool.tile([128, ROWS], bf16, name="trp")
            for s in range(8):
                nc.tensor.transpose(ps[:, s * 128:(s + 1) * 128],
                                    x_t[s][:, t * 128:(t + 1) * 128],
                                    identity_bf[:])
            xt_b = xnt_pool.tile([128, ROWS], bf16, name=f"xnT{t}")
            nc.vector.tensor_copy(xt_b[:], ps[:])
            xnT.append(xt_b)
        tr_stack.close()
        ln_pool.close()

        # ------------------------------------------------------------------
        # Phase 4: QKV projections (bf16) + per-piece kv bounce + AllGathers
        qkv_psum_stack = contextlib.ExitStack()
        qkv_psum = qkv_psum_stack.enter_context(
            tc.tile_pool(name="qkvp", bufs=2, space="PSUM"))
        stage_stack = contextlib.ExitStack()
        stage_pool = stage_stack.enter_context(tc.tile_pool(name="kvstage", bufs=4))

        # v first (every piece needs all of v), then per pair: k piece +
        # AllGather + q projection (so pair 0 can start attention asap).
        qT = []
        for s in range(8):
            ps = qkv_psum.tile([128, DIM], f32, name="qkvps")
            for ki in range(8):
                for nh in range(2):
                    nc.tensor.matmul(ps[:, nh * 512:(nh + 1) * 512],
                                     xnT[ki][:, s * 128:(s + 1) * 128],
                                     wkv_bf[ki][:, DIM + nh * 512:DIM + (nh + 1) * 512],
                                     start=(ki == 0), stop=(ki == 7))
            vst = stage_pool.tile([128, DIM], bf16, name="kvst")
            nc.vector.tensor_copy(vst[:], ps[:])
            # one strided store covering all 8 pair-pieces
            kvl = kv_loc[:]
            nc.sync.dma_start(
                out=bass.AP(tensor=kvl.tensor,
                            offset=kvl.offset + KP + s * 128 * 128,
                            ap=[[128, 128], [PIECE, PAIRS], [1, 128]]),
                in_=vst[:].rearrange("p (i d) -> p i d", i=PAIRS))

        for i in range(PAIRS):
            ps = qkv_psum.tile([128, ROWS], f32, name="qkvps")
            for ki in range(8):
                for nh in range(2):
                    nc.tensor.matmul(ps[:, nh * 512:(nh + 1) * 512],
                                     wkv_bf[ki][:, i * 128:(i + 1) * 128],
                                     xnT[ki][:, nh * 512:(nh + 1) * 512],
                                     start=(ki == 0), stop=(ki == 7))
            kst = stage_pool.tile([128, ROWS], bf16, name="kvst")
            nc.vector.tensor_copy(kst[:], ps[:])
            kvl = kv_loc[:]
            nc.scalar.dma_start(
                out=bass.AP(tensor=kvl.tensor, offset=kvl.offset + i * PIECE,
                            ap=[[DIM, 128], [1, DIM]]),
                in_=kst[:])
            nc.gpsimd.collective_compute(
                "AllGather",
                mybir.AluOpType.bypass,
                replica_groups=[list(range(CORES))],
                ins=[bass.AP(tensor=kvl.tensor, offset=kvl.offset + i * PIECE,
                             ap=[[1, PIECE]]).opt()],
                outs=[kv_ful[i][:].opt()],
            )
            # interleave q projection for this pair
            ps = qkv_psum.tile([128, ROWS], f32, name="qkvps")
            for ki in range(8):
                for nh in range(2):
                    nc.tensor.matmul(ps[:, nh * 512:(nh + 1) * 512],
                                     wq_bf[ki][:, i * 128:(i + 1) * 128],
                                     xnT[ki][:, nh * 512:(nh + 1) * 512],
                                     start=(ki == 0), stop=(ki == 7))
            qtile = qt_pool.tile([128, ROWS], bf16, name=f"qT{i}")
            nc.vector.tensor_scalar_mul(qtile[:], ps[:], SCALE)
            qT.append(qtile)

        stage_stack.close()
        qkv_psum_stack.close()
        xnt_stack.close()
        w_stack.close()

        # ------------------------------------------------------------------
        # Phase 5: attention over head pairs (software-pipelined over chunks)

        attn_stack = contextlib.ExitStack()
        wout_bf = []
        for t in range(8):
            wot = wo_pool.tile([128, DIM], bf16, name=f"wo{t}")
            nc.gpsimd.dma_start(wot[:], wout_in[t * 128:(t + 1) * 128, :])
            wout_bf.append(wot)
        kt_pool = attn_stack.enter_context(tc.tile_pool(name="kT", bufs=2))
        vt_pool = attn_stack.enter_context(tc.tile_pool(name="vt", bufs=2))
        ae_pool = attn_stack.enter_context(tc.tile_pool(name="ae", bufs=8))
        den_pool = attn_stack.enter_context(tc.tile_pool(name="den", bufs=2))
        sim_psum = attn_stack.enter_context(
            tc.tile_pool(name="simp", bufs=2, space="PSUM"))
        out_psum = attn_stack.enter_context(
            tc.tile_pool(name="outp", bufs=2, space="PSUM"))
        dn_psum = attn_stack.enter_context(
            tc.tile_pool(name="dnp", bufs=1, space="PSUM"))
        rb_psum = attn_stack.enter_context(
            tc.tile_pool(name="rbp", bufs=1, space="PSUM"))
        rb_pool = attn_stack.enter_context(tc.tile_pool(name="rbs", bufs=2))

        def normalize_pair(ip):
            dkp = den_keep[ip // 2]
            pip = 64 * (ip % 2)
            for bp in range(2):
                rb_ps = rb_psum.tile([128, 512], f32, name="rbp")
                nc.tensor.matmul(
                    rb_ps[:], sel_full[pip:pip + 33, :],
                    dkp[pip:pip + 33, bp * 512:(bp + 1) * 512],
                    start=True, stop=True)
                rbs = rb_pool.tile([128, 512], f32, name="rbs")
                nc.vector.tensor_copy(rbs[:], rb_ps[:])
                nc.vector.tensor_mul(
                    asm[ip][:, bp * 512:(bp + 1) * 512],
                    asm[ip][:, bp * 512:(bp + 1) * 512], rbs[:])

        for i in range(PAIRS):
            kvf = kv_ful[i][:]
            KVF_T = kvf.tensor
            # k^T for the head pair: [128 (2 heads x 64 d), B*N] bf16
            kt = kt_pool.tile([128, B * N], bf16, name="kt")
            ktd = kt[:]
            for b in range(B):
                nc.sync.dma_start(
                    out=bass.AP(tensor=ktd.tensor, offset=ktd.offset + b * N,
                                ap=[ktd.ap[0], [NLOC, CORES], [1, NLOC]]),
                    in_=bass.AP(tensor=KVF_T,
                                offset=kvf.offset + b * NLOC,
                                ap=[[DIM, 128], [PIECE, CORES], [1, NLOC]]))
            # v for both heads of the pair: [128 tok, (b,c) x 128 inner]
            vt = vt_pool.tile([128, B * KC * 128], bf16, name="vt")
            vb = vt[:]
            for b in range(B):
                for c2 in range(2):
                    nc.gpsimd.dma_start(
                        out=bass.AP(tensor=vb.tensor,
                                    offset=(vb.offset + (b * KC + c2) * 128),
                                    ap=[vb.ap[0], [256, CORES], [1, 128]]),
                        in_=bass.AP(tensor=KVF_T,
                                    offset=(kvf.offset + KP
                                            + (b * NLOC + c2 * 128) * 128),
                                    ap=[[128, 128], [PIECE, CORES], [1, 128]]))
            # prefetch exp(bias^T) two pairs ahead (pairs 0-2 preloaded)
            if 2 * i + 6 < H:
                load_ebt(2 * i + 6)
                load_ebt(2 * i + 7)
            ebt = [ebt_tiles.pop(2 * i), ebt_tiles.pop(2 * i + 1)]

            po, dn = {}, {}
            for bp in range(2):
                p_t = out_psum.tile([128, 512], f32, name="po")
                po[bp] = p_t
                nc.tensor.matmul(p_t[:, :], zeros128[:], qT[i][:, 0:512],
                                 start=True, stop=False, skip_group_check=True)
            d_t = dn_psum.tile([97, 512], f32, name="dn")
            for cg in range(4):
                nc.tensor.matmul(d_t[32 * cg:32 * cg + 1, :],
                                 zeros128[:, 0:1], qT[i][:, 0:512],
                                 start=True, stop=False, skip_group_check=True,
                                 tile_position=(0, 32 * cg))

            ae_ring = {}
            for c in range(KC + 1):
                if c < KC:
                    pss = {}
                    for parity in range(2):
                        pss[parity] = sim_psum.tile([128, B * NLOC], f32,
                                                    name="simps")
                    # interleave parities so consecutive LDWEIGHTS hit
                    # different PE row-groups and overlap the matmuls
                    for b in range(B):
                        for parity in range(2):
                            nc.tensor.matmul(
                                pss[parity][:, b * NLOC:(b + 1) * NLOC],
                                kt[parity * 64:parity * 64 + 64,
                                   b * N + c * 128:b * N + (c + 1) * 128],
                                qT[i][parity * 64:parity * 64 + 64,
                                      b * NLOC:(b + 1) * NLOC],
                                start=True, stop=True,
                                tile_position=(parity * 64, 0))
                    for parity in range(2):
                        ae = ae_pool.tile([128, B * NLOC], bf16, name="ae")
                        nc.scalar.activation(ae[:], pss[parity][:], AF.Exp)
                        ebs = ebt[parity][:, c * NLOC:(c + 1) * NLOC]
                        bcast = bass.AP(tensor=ebs.tensor, offset=ebs.offset,
                                        ap=[ebs.ap[0], [0, B], [1, NLOC]])
                        ae3 = ae[:].rearrange("p (b q) -> p b q", b=B)
                        nc.vector.tensor_tensor(out=ae3, in0=ae3, in1=bcast,
                                                op=ALU.mult)
                        ae_ring[c, parity] = ae
                if c >= 1:
                    cp = c - 1
                    ae_e = ae_ring.pop((cp, 0))
                    ae_o = ae_ring.pop((cp, 1))
                    for b in range(B):
                        blk = (b * KC + cp) * 128
                        # attn@v: head-even -> PE cols 0-63, head-odd ->
                        # cols 64-127; the two matmuls co-execute
                        nc.tensor.matmul(
                            po[b // 2][0:64, (b % 2) * NLOC:((b % 2) + 1) * NLOC],
                            vt[:, blk:blk + 64],
                            ae_e[:, b * NLOC:(b + 1) * NLOC],
                            start=False, stop=(cp == KC - 1),
                            tile_position=(0, 0), skip_group_check=True)
                        nc.tensor.matmul(
                            po[b // 2][64:128, (b % 2) * NLOC:((b % 2) + 1) * NLOC],
                            vt[:, blk + 64:blk + 128],
                            ae_o[:, b * NLOC:(b + 1) * NLOC],
                            start=False, stop=(cp == KC - 1),
                            tile_position=(0, 64), skip_group_check=True)
                    # denominators: 4-way column-tiled ones-weight matmuls
                    for cg, (ae_t, bp) in enumerate(
                            [(ae_e, 0), (ae_o, 0), (ae_e, 1), (ae_o, 1)]):
                        nc.tensor.matmul(
                            d_t[32 * cg:32 * cg + 1, :], ones1[:],
                            ae_t[:, bp * 512:(bp + 1) * 512],
                            start=False, stop=(cp == KC - 1),
                            tile_position=(0, 32 * cg), skip_group_check=True)

            # per-pair normalize: recip(den) -> broadcast -> asm = po * recip
            # den_row rows 0 (head-even) / 32 (head-odd); cols 0-1023 den,
            # 1024-2047 recip
            # evacuate pair outputs (unnormalized) + stash denominators;
            # normalization happens in the tail, off the inter-pair path
            dk = den_keep[i // 2]
            pi = 64 * (i % 2)
            for bp in range(2):
                nc.vector.tensor_copy(
                    asm[i][:, bp * 512:(bp + 1) * 512], po[bp][:, :])
            for cg, (par, bp) in enumerate([(0, 0), (1, 0), (0, 1), (1, 1)]):
                nc.vector.tensor_copy(
                    dk[pi + 32 * par:pi + 32 * par + 1,
                       bp * 512:(bp + 1) * 512],
                    d_t[32 * cg:32 * cg + 1, :])
            nc.vector.reciprocal(dk[pi:pi + 33, :], dk[pi:pi + 33, :])
            if i >= 1:
                normalize_pair(i - 1)

        normalize_pair(PAIRS - 1)

        attn_stack.close()
        ebt_stack.close()

        # ------------------------------------------------------------------
        # Phase 7: output projection
        fin_stack = contextlib.ExitStack()
        f_psum = fin_stack.enter_context(
            tc.tile_pool(name="fp", bufs=2, space="PSUM"))
        o_pool = fin_stack.enter_context(tc.tile_pool(name="osb", bufs=3))
        for mi in range(8):
            ps = f_psum.tile([128, DIM], f32, name="fp")
            for ki in range(8):
                for nh in range(2):
                    nc.tensor.matmul(ps[:, nh * 512:(nh + 1) * 512],
                                     asm[ki][:, mi * 128:(mi + 1) * 128],
                                     wout_bf[ki][:, nh * 512:(nh + 1) * 512],
                                     start=(ki == 0), stop=(ki == 7))
            ot = o_pool.tile([128, DIM], f32, name="ot")
            nc.vector.tensor_copy(ot[:], ps[:])
            nc.sync.dma_start(out_ext[mi * 128:(mi + 1) * 128, :], ot[:])

        fin_stack.close()

    nc.finalize()
    return nc


def _get_nc():
    if "nc" not in _CACHE:
        _CACHE["nc"] = _build_nc()
    return _CACHE["nc"]


def prep_in_maps(x, attn_bias, ln_g, ln_b, Wq, Wkv, Wout):
    """Host-side sharding + layout prep (slice/transpose/cast only)."""
    import ml_dtypes

    bf16 = ml_dtypes.bfloat16
    x = np.asarray(x, dtype=np.float32)
    attn_bias = np.asarray(attn_bias, dtype=np.float32)
    wq = np.asarray(Wq, dtype=np.float32).astype(bf16)
    wkv = np.asarray(Wkv, dtype=np.float32).astype(bf16)
    wout = np.asarray(Wout, dtype=np.float32).astype(bf16)
    ln_g = np.asarray(ln_g, dtype=np.float32).astype(bf16)
    ln_b = np.asarray(ln_b, dtype=np.float32).astype(bf16)
    in_maps = []
    for r in range(CORES):
        # bias slice [H, 256 q, 2048 kv] -> kv-major [H, 128 p, KC, 256 q]
        bslice = attn_bias[:, r * NLOC:(r + 1) * NLOC, :]
        bT = np.ascontiguousarray(
            bslice.reshape(H, NLOC, KC, 128).transpose(0, 3, 2, 1)
        ).astype(bf16).reshape(H, 128, KC * NLOC)
        in_maps.append({
            "x": np.ascontiguousarray(
                x[:, r * NLOC:(r + 1) * NLOC, :]).reshape(ROWS, DIM).astype(bf16),
            "attn_bias": bT,
            "ln_g": ln_g,
            "ln_b": ln_b,
            "Wq": wq,
            "Wkv": wkv,
            "Wout": wout,
        })
    return in_maps


def kernel(x, attn_bias, ln_g, ln_b, Wq, Wkv, Wout):
    from concourse import bass_utils

    nc = _get_nc()
    in_maps = prep_in_maps(x, attn_bias, ln_g, ln_b, Wq, Wkv, Wout)
    res = bass_utils.run_bass_kernel_spmd(nc, in_maps, core_ids=list(range(CORES)))
    out = np.empty((B, N, DIM), dtype=np.float32)
    for r in range(CORES):
        out[:, r * NLOC:(r + 1) * NLOC, :] = \
            res.results[r]["out"].reshape(B, NLOC, DIM)
    return out
